# revision 22
# baseline (speedup 1.0000x reference)
"""GCN message-passing kernel for Trainium2, 8 NeuronCores.

Net: 4x { h -> relu(segment_sum(h[src], dst) @ W + b) } (no relu on last).
N=100000 nodes, E=3200000 edges, dims 256->256->256->128->2.

Strategy (pull-model SpMM, transposed accumulation):
  - dst-nodes block-partitioned across 8 cores (12500/core, padded to 12544).
  - Node features live in a "table order": 7 chunks x (8 ranks x 1792 rows),
    so each chunked AllGather writes a contiguous table slice.
  - Blocks are processed in groups of G=3; per group one gpsimd.dma_gather
    per table quadrant (int16 gather indices limit the addressable window to
    32k rows -> 4 quadrants) pulls the group's edge src rows (bf16) to SBUF.
  - Per 128-dst block: one-hot edge->slot indicators on DVE (is_equal vs
    iota), then PE accumulates the TRANSPOSED aggregate directly:
    accT[f,d] += msgs_tile[e,f]^T @ ind[e,d].  The layer linear then reads
    accT as lhsT with no PE transposes: h[d,:] = sum_k accT_k^T @ W[k half].
  - Aggregation commutes with the linear, so layer k applies W_k after
    aggregating h_{k-1}; only h tables are exchanged (no pre-transformed
    tables).  The same edge schedule drives all 4 layers.
  - Emission is software-pipelined: group g's agg matmuls are emitted before
    group g-1's per-block linear tails so the in-order engines never stall
    on cross-engine round trips.
"""
import sys
sys.path.insert(0, '/opt/trn_rl_repo')

import numpy as np
import ml_dtypes

from concourse import bass, mybir, tile, bacc
from concourse import bass_utils

dt = mybir.dt
bf16 = ml_dtypes.bfloat16

# ---------------- problem constants (hardcoded per spec) ----------------
N, E, D = 100000, 3200000, 256
OUTD = 2
N_CORES = 8
RPC = 12500                 # real nodes per core
NCHUNK = 7
BLK = 128
NBLK = 98
NLOC = NBLK * BLK           # 12544 padded rows per core
CHUNK = NLOC // NCHUNK      # 1792 rows per rank per chunk
BPC = NBLK // NCHUNK        # 14 blocks per chunk
NTAB = N_CORES * NLOC       # 100352 table rows
QROWS = NTAB // 4           # 25088 (< int16 max)
PAD_SLOT = 200.0
G = 3                       # dst blocks per gather group
NGRP = (NBLK + G - 1) // G  # 33 groups (last has 2 blocks)


def _table_id(v):
    r = v // RPC
    l = v % RPC
    c = l // CHUNK
    p = l % CHUNK
    return c * (N_CORES * CHUNK) + r * CHUNK + p


def preprocess(src, dst):
    """Build per-core gather/slot streams and shared segment-size tables.

    Stream layout: for g in groups: for q in quadrants: for b in g's blocks:
    that (b,q)'s edges padded to a multiple of 128.  One dma_gather covers a
    whole (g,q) range; tiles stay (b,q)-pure so the one-hot indicator logic
    is per 128-edge tile.
    """
    tid_src = _table_id(src.astype(np.int64))
    core = dst // RPC
    dl = dst % RPC
    blk = dl // BLK
    slot = dl % BLK
    q = tid_src // QROWS
    qidx = (tid_src % QROWS).astype(np.int32)

    # group edges by (core, blk, q)
    cnt = np.zeros((N_CORES, NBLK, 4), np.int64)
    order = np.lexsort((q, blk, core))
    core_s, blk_s, q_s = core[order], blk[order], q[order]
    qidx_s, slot_s = qidx[order], slot[order]
    np.add.at(cnt, (core_s, blk_s, q_s), 1)

    num_reg = cnt.max(axis=0)                         # [NBLK, 4]
    num_idx = ((num_reg + 127) // 128) * 128          # [NBLK, 4]
    nt_bq = num_idx // 128                            # tiles per (b, q)
    nt_blk = nt_bq.sum(axis=1)                        # tiles per block
    assert (num_reg > 0).all()

    # stream layout: (g, q, b-within-g)
    seg_off = np.zeros((NBLK, 4), np.int64)   # idx-stream offset of (b, q)
    tile_of = np.zeros((NBLK, 4), np.int64)   # global tile index of (b, q)
    grp_blocks = [list(range(g * G, min((g + 1) * G, NBLK)))
                  for g in range(NGRP)]
    gq_off = np.zeros((NGRP, 4), np.int64)    # idx offset of (g, q) segment
    gq_len = np.zeros((NGRP, 4), np.int64)
    gq_t0 = np.zeros((NGRP, 4), np.int64)     # first tile of (g, q) segment
    g_off = np.zeros(NGRP, np.int64)          # idx offset of group
    g_len = np.zeros(NGRP, np.int64)
    g_t0 = np.zeros(NGRP, np.int64)           # first tile of group
    o = t = 0
    for g in range(NGRP):
        g_off[g] = o
        g_t0[g] = t
        for qq in range(4):
            gq_off[g, qq] = o
            gq_t0[g, qq] = t
            for b in grp_blocks[g]:
                seg_off[b, qq] = o
                tile_of[b, qq] = t
                o += num_idx[b, qq]
                t += nt_bq[b, qq]
            gq_len[g, qq] = o - gq_off[g, qq]
        g_len[g] = o - g_off[g]
    tot_idx = o
    tot_tiles = t

    # per-core edge stream bounds in the (core, blk, q)-sorted order
    bounds = np.zeros((N_CORES, NBLK, 4, 2), np.int64)
    pos = 0
    for c in range(N_CORES):
        for b in range(NBLK):
            for qq in range(4):
                n = cnt[c, b, qq]
                bounds[c, b, qq] = (pos, pos + n)
                pos += n

    idx_streams, slot_streams, tid_streams = [], [], []
    for c in range(N_CORES):
        idx_s = np.zeros(tot_idx, np.int16)
        tid_s = np.zeros(tot_idx, np.int32)
        slt_s = np.full(tot_tiles * 128, PAD_SLOT, np.float32)
        for b in range(NBLK):
            for qq in range(4):
                lo, hi = bounds[c, b, qq]
                n = hi - lo
                o0 = seg_off[b, qq]
                idx_s[o0:o0 + n] = qidx_s[lo:hi]
                tid_s[o0:o0 + n] = qq * QROWS + qidx_s[lo:hi]
                t0 = tile_of[b, qq] * 128
                slt_s[t0:t0 + n] = slot_s[lo:hi]
        # wrap idx: position i -> [i % 16, i // 16], replicate to 128 partitions
        idx_w = idx_s.reshape(-1, 16).T
        idx_streams.append(np.tile(idx_w, (8, 1)))
        tid_streams.append(tid_s)
        # slots: edge j of tile t -> [j, t]
        slot_streams.append(slt_s.reshape(tot_tiles, 128).T.astype(bf16))

    meta = dict(num_idx=num_idx, nt_bq=nt_bq, nt_blk=nt_blk,
                seg_off=seg_off, tile_of=tile_of,
                grp_blocks=grp_blocks, gq_off=gq_off, gq_len=gq_len,
                gq_t0=gq_t0, g_off=g_off, g_len=g_len, g_t0=g_t0,
                tot_idx=tot_idx, tot_tiles=tot_tiles)
    return meta, idx_streams, slot_streams, tid_streams


def build_program(meta, repeat=1, layers=4, collectives=True):
    num_idx, nt_bq, nt_blk = meta['num_idx'], meta['nt_bq'], meta['nt_blk']
    tile_of = meta['tile_of']
    grp_blocks = meta['grp_blocks']
    gq_off, gq_len, gq_t0 = meta['gq_off'], meta['gq_len'], meta['gq_t0']
    g_off, g_len, g_t0 = meta['g_off'], meta['g_len'], meta['g_t0']
    tot_idx, tot_tiles = meta['tot_idx'], meta['tot_tiles']

    nc = bacc.Bacc("TRN2", target_bir_lowering=False, debug=False,
                   num_devices=N_CORES)

    x_edges_d = nc.dram_tensor("x_edges", [128, tot_tiles, D], dt.bfloat16,
                               kind="ExternalInput")
    idxs_d = nc.dram_tensor("idxs", [128, tot_idx // 16], dt.int16, kind="ExternalInput")
    slots_d = nc.dram_tensor("slots", [128, tot_tiles], dt.bfloat16, kind="ExternalInput")
    iota_d = nc.dram_tensor("iota", [128, 128], dt.bfloat16, kind="ExternalInput")
    w1_d = nc.dram_tensor("w1", [256, 256], dt.bfloat16, kind="ExternalInput")
    w2_d = nc.dram_tensor("w2", [256, 256], dt.bfloat16, kind="ExternalInput")
    w3_d = nc.dram_tensor("w3", [256, 128], dt.bfloat16, kind="ExternalInput")
    w4_d = nc.dram_tensor("w4", [128, 2], dt.bfloat16, kind="ExternalInput")
    b1_d = nc.dram_tensor("b1", [1, 256], dt.bfloat16, kind="ExternalInput")
    b2_d = nc.dram_tensor("b2", [1, 256], dt.bfloat16, kind="ExternalInput")
    b3_d = nc.dram_tensor("b3", [1, 128], dt.bfloat16, kind="ExternalInput")
    b4_d = nc.dram_tensor("b4", [1, 2], dt.bfloat16, kind="ExternalInput")
    tok_d = nc.dram_tensor("tok", [128, 4], dt.float32, kind="ExternalInput")

    outp = nc.dram_tensor("outp", [2, NLOC], dt.float32, kind="ExternalOutput")
    tok_out = nc.dram_tensor("tok_out", [128, 4], dt.float32, kind="ExternalOutput")

    h1_tab = nc.dram_tensor("h1_tab", [NTAB, D], dt.bfloat16, addr_space="Shared")
    h2_tab = nc.dram_tensor("h2_tab", [NTAB, D], dt.bfloat16, addr_space="Shared")
    h3_tab = nc.dram_tensor("h3_tab", [NTAB, 128], dt.bfloat16, addr_space="Shared")
    sh1 = [nc.dram_tensor(f"sh1_{c}", [CHUNK, D], dt.bfloat16) for c in range(NCHUNK)]
    sh2 = [nc.dram_tensor(f"sh2_{c}", [CHUNK, D], dt.bfloat16) for c in range(NCHUNK)]
    sh3 = [nc.dram_tensor(f"sh3_{c}", [CHUNK, 128], dt.bfloat16) for c in range(NCHUNK)]

    RG = [list(range(N_CORES))]
    AF = mybir.ActivationFunctionType

    with tile.TileContext(nc) as tc:
        with tc.tile_pool(name="const", bufs=1) as cpool, \
             tc.tile_pool(name="msgp", bufs=2) as msgp, \
             tc.tile_pool(name="idxp", bufs=2) as idxp, \
             tc.tile_pool(name="indp", bufs=4) as indp, \
             tc.tile_pool(name="accp", bufs=8) as accp, \
             tc.tile_pool(name="work", bufs=4) as work, \
             tc.tile_pool(name="psA", bufs=4, space="PSUM") as psA, \
             tc.tile_pool(name="psB", bufs=2, space="PSUM") as psB, \
             tc.tile_pool(name="psO", bufs=2, space="PSUM") as psO:

            slot_all = cpool.tile([128, tot_tiles], dt.bfloat16)
            nc.sync.dma_start(out=slot_all[:], in_=slots_d[:])
            iota_t = cpool.tile([128, 128], dt.bfloat16)
            nc.sync.dma_start(out=iota_t[:], in_=iota_d[:])
            w1_t = cpool.tile([128, 2, 256], dt.bfloat16)
            nc.sync.dma_start(out=w1_t[:, 0, :], in_=w1_d[0:128, :])
            nc.sync.dma_start(out=w1_t[:, 1, :], in_=w1_d[128:256, :])
            w2_t = cpool.tile([128, 2, 256], dt.bfloat16)
            nc.sync.dma_start(out=w2_t[:, 0, :], in_=w2_d[0:128, :])
            nc.sync.dma_start(out=w2_t[:, 1, :], in_=w2_d[128:256, :])
            w3_t = cpool.tile([128, 2, 128], dt.bfloat16)
            nc.sync.dma_start(out=w3_t[:, 0, :], in_=w3_d[0:128, :])
            nc.sync.dma_start(out=w3_t[:, 1, :], in_=w3_d[128:256, :])
            w4_t = cpool.tile([128, 2], dt.bfloat16)
            nc.sync.dma_start(out=w4_t[:], in_=w4_d[:])
            b1_t = cpool.tile([1, 256], dt.bfloat16)
            nc.sync.dma_start(out=b1_t[:], in_=b1_d[:])
            b2_t = cpool.tile([1, 256], dt.bfloat16)
            nc.sync.dma_start(out=b2_t[:], in_=b2_d[:])
            b3_t = cpool.tile([1, 128], dt.bfloat16)
            nc.sync.dma_start(out=b3_t[:], in_=b3_d[:])
            b4_t = cpool.tile([1, 2], dt.bfloat16)
            nc.sync.dma_start(out=b4_t[:], in_=b4_d[:])
            ones_t = cpool.tile([1, 128], dt.bfloat16)
            nc.vector.memset(ones_t[:], 1.0)
            tok_t = cpool.tile([128, 4], dt.float32)
            nc.sync.dma_start(out=tok_t[:], in_=tok_d[:])

            def gather_group(g, tab_ap, Dl):
                """Gather all edges of group g: msgs [128, g_tiles, Dl]."""
                gt0 = int(g_t0[g])
                gtiles = int(g_len[g]) // 128
                cols = int(g_len[g]) // 16
                c0 = int(g_off[g]) // 16
                idxg = idxp.tile([128, cols], dt.int16, tag="idxg")
                nc.sync.dma_start(out=idxg[:], in_=idxs_d[:, c0:c0 + cols])
                msgs = msgp.tile([128, gtiles, Dl], dt.bfloat16, tag="msgs")
                for qq in range(4):
                    ni = int(gq_len[g, qq])
                    if ni == 0:
                        continue
                    o16 = int(gq_off[g, qq] - g_off[g]) // 16
                    to = int(gq_t0[g, qq]) - gt0
                    ntq = ni // 128
                    nc.gpsimd.dma_gather(
                        msgs[:, to:to + ntq, :],
                        tab_ap[qq * QROWS:(qq + 1) * QROWS, :],
                        idxg[:, o16:o16 + ni // 16],
                        ni, ni, Dl, single_packet=False)
                return msgs, gt0

            def block_pairs(b, gt0):
                """(group-tile index, block-tile index) pairs for block b."""
                pairs = []
                bo = 0
                for qq in range(4):
                    ntq = int(nt_bq[b, qq])
                    gt = int(tile_of[b, qq]) - gt0
                    for k in range(ntq):
                        pairs.append((gt + k, bo + k))
                    bo += ntq
                return pairs

            def block_ind(b):
                """One-hot indicators for block b's tiles."""
                ntb = int(nt_blk[b])
                ind = indp.tile([128, ntb, 128], dt.bfloat16, tag="ind")
                bo = 0
                for qq in range(4):
                    ntq = int(nt_bq[b, qq])
                    if ntq == 0:
                        continue
                    t0 = int(tile_of[b, qq])
                    nc.vector.tensor_tensor(
                        out=ind[:, bo:bo + ntq, :],
                        in0=slot_all[:, t0:t0 + ntq][:, :, None]
                            .to_broadcast([128, ntq, 128]),
                        in1=iota_t[:][:, None, :].to_broadcast([128, ntq, 128]),
                        op=mybir.AluOpType.is_equal)
                    bo += ntq
                return ind

            def agg_blockT(b, msgs, gt0, nhalf):
                """accT[f_half, k, d] += msgs^T @ ind, one chain per half."""
                ind = block_ind(b)
                pairs = block_pairs(b, gt0)
                acc = psA.tile([128, nhalf, 128], dt.float32, space="PSUM",
                               tag="acc")
                for k in range(nhalf):
                    for i, (mt, it) in enumerate(pairs):
                        nc.tensor.matmul(
                            out=acc[:, k, :],
                            lhsT=msgs[:, mt, k * 128:(k + 1) * 128],
                            rhs=ind[:, it, :],
                            start=(i == 0), stop=(i == len(pairs) - 1))
                return acc

            def tail_linear(b, acc, nhalf, w_t, b_t, nout, relu, h_shards,
                            tab):
                """h[d, nout] = (relu?)(sum_k accT_k^T @ W_k + b); store."""
                a_sb = accp.tile([128, nhalf, 128], dt.bfloat16, tag="a_sb")
                for k in range(nhalf):
                    nc.vector.tensor_copy(out=a_sb[:, k, :], in_=acc[:, k, :])
                hps = psB.tile([128, nout], dt.float32, space="PSUM", tag="h")
                for k in range(nhalf):
                    nc.tensor.matmul(out=hps[:], lhsT=a_sb[:, k, :],
                                     rhs=w_t[:, k, :], start=(k == 0),
                                     stop=False)
                nc.tensor.matmul(out=hps[:], lhsT=ones_t[:], rhs=b_t[:],
                                 start=False, stop=True)
                h_blk = work.tile([128, nout], dt.bfloat16, tag="hblk")
                if relu:
                    nc.scalar.activation(out=h_blk[:], in_=hps[:], func=AF.Relu)
                else:
                    nc.vector.tensor_copy(out=h_blk[:], in_=hps[:])
                r0 = (b % BPC) * BLK
                nc.sync.dma_start(out=h_shards[b // BPC][r0:r0 + BLK, :],
                                  in_=h_blk[:])
                if collectives and b % BPC == BPC - 1:
                    c = b // BPC
                    nc.gpsimd.collective_compute(
                        "AllGather", mybir.AluOpType.bypass, replica_groups=RG,
                        ins=[h_shards[c][:]],
                        outs=[tab[c * N_CORES * CHUNK:(c + 1) * N_CORES * CHUNK, :]])

            def tail_out(b, acc):
                """out rows = W4^T @ accT + b4; store [2, 128] columns."""
                a_sb = accp.tile([128, 1, 128], dt.bfloat16, tag="a_sb")
                nc.vector.tensor_copy(out=a_sb[:, 0, :], in_=acc[:, 0, :])
                ops = psO.tile([2, 128], dt.float32, space="PSUM", tag="o")
                nc.tensor.matmul(out=ops[:], lhsT=w4_t[:], rhs=a_sb[:, 0, :],
                                 start=True, stop=False)
                nc.tensor.matmul(out=ops[:], lhsT=b4_t[:], rhs=ones_t[:],
                                 start=False, stop=True)
                ob = work.tile([2, 128], dt.float32, tag="ob")
                nc.vector.tensor_copy(out=ob[:], in_=ops[:])
                nc.sync.dma_start(out=outp[:, b * BLK:(b + 1) * BLK], in_=ob[:])

            def stream_group(g, Dl):
                """L1: bulk-load the host-pregathered edge stream slice."""
                gt0 = int(g_t0[g])
                gtiles = int(g_len[g]) // 128
                msgs = msgp.tile([128, gtiles, Dl], dt.bfloat16, tag="msgs")
                nc.sync.dma_start(out=msgs[:],
                                  in_=x_edges_d[:, gt0:gt0 + gtiles, :])
                return msgs, gt0

            def run_layer(tab_in, Dl, tail_fn, stream=False):
                """Software-pipelined: aggs of group g, then tails of g-1."""
                nhalf = Dl // 128
                pending = []
                for g in range(NGRP):
                    if stream:
                        msgs, gt0 = stream_group(g, Dl)
                    else:
                        msgs, gt0 = gather_group(g, tab_in, Dl)
                    new_pending = []
                    for b in grp_blocks[g]:
                        acc = agg_blockT(b, msgs, gt0, nhalf)
                        new_pending.append((b, acc))
                    for b, acc in pending:
                        tail_fn(b, acc)
                    pending = new_pending
                for b, acc in pending:
                    tail_fn(b, acc)

            for rep in range(repeat):
                run_layer(None, D, lambda b, acc: tail_linear(
                    b, acc, 2, w1_t, b1_t, 256, True, sh1, h1_tab), stream=True)
                if layers >= 2:
                    run_layer(h1_tab, D, lambda b, acc: tail_linear(
                        b, acc, 2, w2_t, b2_t, 256, True, sh2, h2_tab))
                if layers >= 3:
                    run_layer(h2_tab, D, lambda b, acc: tail_linear(
                        b, acc, 2, w3_t, b3_t, 128, True, sh3, h3_tab))
                if layers >= 4:
                    run_layer(h3_tab, 128, tail_out)

            # token passthrough (anti-CSE for timing harness)
            nc.scalar.mul(tok_t[:], tok_t[:], 2.0)
            nc.sync.dma_start(out=tok_out[:], in_=tok_t[:])

    nc.compile()
    return nc


def make_in_maps(x, src, dst, W1, b1, W2, b2, W3, b3, W4, b4,
                 meta, idx_streams, slot_streams, tid_streams):
    v = np.arange(N)
    tid = _table_id(v)
    x_tab = np.zeros((NTAB, D), bf16)
    x_tab[tid] = np.asarray(x).astype(bf16)
    tot_tiles = int(meta['tot_tiles'])
    iota_np = np.tile(np.arange(128, dtype=np.float32)[None, :], (128, 1)).astype(bf16)
    common = {
        "iota": iota_np,
        "w1": np.asarray(W1).astype(bf16), "w2": np.asarray(W2).astype(bf16),
        "w3": np.asarray(W3).astype(bf16), "w4": np.asarray(W4).astype(bf16),
        "b1": np.asarray(b1).reshape(1, -1).astype(bf16),
        "b2": np.asarray(b2).reshape(1, -1).astype(bf16),
        "b3": np.asarray(b3).reshape(1, -1).astype(bf16),
        "b4": np.asarray(b4).reshape(1, -1).astype(bf16),
        "tok": np.zeros((128, 4), np.float32),
    }
    in_maps = []
    for c in range(N_CORES):
        m = dict(common)
        m["idxs"] = idx_streams[c]
        m["slots"] = slot_streams[c]
        # host-pregathered layer-1 edge stream: position i -> [i%128, i//128]
        m["x_edges"] = x_tab[tid_streams[c].reshape(tot_tiles, 128).T]
        in_maps.append(m)
    return in_maps


def assemble_output(results):
    out = np.zeros((N, OUTD), np.float32)
    for c in range(N_CORES):
        o = results[c]["outp"]            # [2, NLOC]
        out[c * RPC:(c + 1) * RPC, :] = o.T[:RPC, :]
    return out


_CACHE = {}
LAST = {}


def kernel(x, src, dst, W1, b1, W2, b2, W3, b3, W4, b4):
    src = np.asarray(src)
    dst = np.asarray(dst)
    key = (src.tobytes(), dst.tobytes())
    kh = hash(key)
    if kh in _CACHE:
        meta, idx_streams, slot_streams, tid_streams, nc = _CACHE[kh]
    else:
        meta, idx_streams, slot_streams, tid_streams = preprocess(src, dst)
        nc = build_program(meta)
        _CACHE[kh] = (meta, idx_streams, slot_streams, tid_streams, nc)
    in_maps = make_in_maps(x, src, dst, W1, b1, W2, b2, W3, b3, W4, b4,
                           meta, idx_streams, slot_streams, tid_streams)
    LAST.update(nc=nc, in_maps=in_maps, meta=meta)
    res = bass_utils.run_bass_kernel_spmd(nc, in_maps, core_ids=list(range(N_CORES)))
    return assemble_output(res.results)


# revision 23
# speedup vs baseline: 1.0180x; 1.0180x over previous
"""GCN message-passing kernel for Trainium2, 8 NeuronCores.

Net: 4x { h -> relu(segment_sum(h[src], dst) @ W + b) } (no relu on last).
N=100000 nodes, E=3200000 edges, dims 256->256->256->128->2.

Strategy (pull-model SpMM, transposed accumulation):
  - dst-nodes block-partitioned across 8 cores (12500/core, padded to 12544).
  - Node features live in a "table order": 7 chunks x (8 ranks x 1792 rows),
    so each chunked AllGather writes a contiguous table slice.
  - Blocks are processed in groups of G=3; per group one gpsimd.dma_gather
    per table quadrant (int16 gather indices limit the addressable window to
    32k rows -> 4 quadrants) pulls the group's edge src rows (bf16) to SBUF.
  - Per 128-dst block: one-hot edge->slot indicators on DVE (is_equal vs
    iota), then PE accumulates the TRANSPOSED aggregate directly:
    accT[f,d] += msgs_tile[e,f]^T @ ind[e,d].  The layer linear then reads
    accT as lhsT with no PE transposes: h[d,:] = sum_k accT_k^T @ W[k half].
  - Aggregation commutes with the linear, so layer k applies W_k after
    aggregating h_{k-1}; only h tables are exchanged (no pre-transformed
    tables).  The same edge schedule drives all 4 layers.
  - Emission is software-pipelined: group g's agg matmuls are emitted before
    group g-1's per-block linear tails so the in-order engines never stall
    on cross-engine round trips.
"""
import sys
sys.path.insert(0, '/opt/trn_rl_repo')

import numpy as np
import ml_dtypes

from concourse import bass, mybir, tile, bacc
from concourse import bass_utils

dt = mybir.dt
bf16 = ml_dtypes.bfloat16

# ---------------- problem constants (hardcoded per spec) ----------------
N, E, D = 100000, 3200000, 256
OUTD = 2
N_CORES = 8
RPC = 12500                 # real nodes per core
NCHUNK = 7
BLK = 128
NBLK = 98
NLOC = NBLK * BLK           # 12544 padded rows per core
CHUNK = NLOC // NCHUNK      # 1792 rows per rank per chunk
BPC = NBLK // NCHUNK        # 14 blocks per chunk
NTAB = N_CORES * NLOC       # 100352 table rows
QROWS = NTAB // 4           # 25088 (< int16 max)
PAD_SLOT = 200.0
G = 3                       # dst blocks per gather group
NGRP = (NBLK + G - 1) // G  # 33 groups (last has 2 blocks)


def _table_id(v):
    r = v // RPC
    l = v % RPC
    c = l // CHUNK
    p = l % CHUNK
    return c * (N_CORES * CHUNK) + r * CHUNK + p


def preprocess(src, dst):
    """Build per-core gather/slot streams and shared segment-size tables.

    Stream layout: for g in groups: for q in quadrants: for b in g's blocks:
    that (b,q)'s edges padded to a multiple of 128.  One dma_gather covers a
    whole (g,q) range; tiles stay (b,q)-pure so the one-hot indicator logic
    is per 128-edge tile.
    """
    tid_src = _table_id(src.astype(np.int64))
    core = dst // RPC
    dl = dst % RPC
    blk = dl // BLK
    slot = dl % BLK
    q = tid_src // QROWS
    qidx = (tid_src % QROWS).astype(np.int32)

    # group edges by (core, blk, q)
    cnt = np.zeros((N_CORES, NBLK, 4), np.int64)
    order = np.lexsort((q, blk, core))
    core_s, blk_s, q_s = core[order], blk[order], q[order]
    qidx_s, slot_s = qidx[order], slot[order]
    np.add.at(cnt, (core_s, blk_s, q_s), 1)

    num_reg = cnt.max(axis=0)                         # [NBLK, 4]
    num_idx = ((num_reg + 127) // 128) * 128          # [NBLK, 4]
    nt_bq = num_idx // 128                            # tiles per (b, q)
    nt_blk = nt_bq.sum(axis=1)                        # tiles per block
    assert (num_reg > 0).all()

    # stream layout: (g, q, b-within-g)
    seg_off = np.zeros((NBLK, 4), np.int64)   # idx-stream offset of (b, q)
    tile_of = np.zeros((NBLK, 4), np.int64)   # global tile index of (b, q)
    grp_blocks = [list(range(g * G, min((g + 1) * G, NBLK)))
                  for g in range(NGRP)]
    gq_off = np.zeros((NGRP, 4), np.int64)    # idx offset of (g, q) segment
    gq_len = np.zeros((NGRP, 4), np.int64)
    gq_t0 = np.zeros((NGRP, 4), np.int64)     # first tile of (g, q) segment
    g_off = np.zeros(NGRP, np.int64)          # idx offset of group
    g_len = np.zeros(NGRP, np.int64)
    g_t0 = np.zeros(NGRP, np.int64)           # first tile of group
    o = t = 0
    for g in range(NGRP):
        g_off[g] = o
        g_t0[g] = t
        for qq in range(4):
            gq_off[g, qq] = o
            gq_t0[g, qq] = t
            for b in grp_blocks[g]:
                seg_off[b, qq] = o
                tile_of[b, qq] = t
                o += num_idx[b, qq]
                t += nt_bq[b, qq]
            gq_len[g, qq] = o - gq_off[g, qq]
        g_len[g] = o - g_off[g]
    tot_idx = o
    tot_tiles = t

    # per-core edge stream bounds in the (core, blk, q)-sorted order
    bounds = np.zeros((N_CORES, NBLK, 4, 2), np.int64)
    pos = 0
    for c in range(N_CORES):
        for b in range(NBLK):
            for qq in range(4):
                n = cnt[c, b, qq]
                bounds[c, b, qq] = (pos, pos + n)
                pos += n

    idx_streams, slot_streams, tid_streams = [], [], []
    for c in range(N_CORES):
        idx_s = np.zeros(tot_idx, np.int16)
        tid_s = np.zeros(tot_idx, np.int32)
        slt_s = np.full(tot_tiles * 128, PAD_SLOT, np.float32)
        for b in range(NBLK):
            for qq in range(4):
                lo, hi = bounds[c, b, qq]
                n = hi - lo
                o0 = seg_off[b, qq]
                # ascending table rows within the sub-segment: better HBM
                # page locality for the gather (order is aggregation-neutral)
                so = np.argsort(qidx_s[lo:hi], kind='stable')
                idx_s[o0:o0 + n] = qidx_s[lo:hi][so]
                tid_s[o0:o0 + n] = qq * QROWS + qidx_s[lo:hi][so]
                t0 = tile_of[b, qq] * 128
                slt_s[t0:t0 + n] = slot_s[lo:hi][so]
        # wrap idx: position i -> [i % 16, i // 16], replicate to 128 partitions
        idx_w = idx_s.reshape(-1, 16).T
        idx_streams.append(np.tile(idx_w, (8, 1)))
        tid_streams.append(tid_s)
        # slots: edge j of tile t -> [j, t]
        slot_streams.append(slt_s.reshape(tot_tiles, 128).T.astype(bf16))

    meta = dict(num_idx=num_idx, nt_bq=nt_bq, nt_blk=nt_blk,
                seg_off=seg_off, tile_of=tile_of,
                grp_blocks=grp_blocks, gq_off=gq_off, gq_len=gq_len,
                gq_t0=gq_t0, g_off=g_off, g_len=g_len, g_t0=g_t0,
                tot_idx=tot_idx, tot_tiles=tot_tiles)
    return meta, idx_streams, slot_streams, tid_streams


def build_program(meta, repeat=1, layers=4, collectives=True):
    num_idx, nt_bq, nt_blk = meta['num_idx'], meta['nt_bq'], meta['nt_blk']
    tile_of = meta['tile_of']
    grp_blocks = meta['grp_blocks']
    gq_off, gq_len, gq_t0 = meta['gq_off'], meta['gq_len'], meta['gq_t0']
    g_off, g_len, g_t0 = meta['g_off'], meta['g_len'], meta['g_t0']
    tot_idx, tot_tiles = meta['tot_idx'], meta['tot_tiles']

    nc = bacc.Bacc("TRN2", target_bir_lowering=False, debug=False,
                   num_devices=N_CORES)

    x_edges_d = nc.dram_tensor("x_edges", [128, tot_tiles, D], dt.bfloat16,
                               kind="ExternalInput")
    idxs_d = nc.dram_tensor("idxs", [128, tot_idx // 16], dt.int16, kind="ExternalInput")
    slots_d = nc.dram_tensor("slots", [128, tot_tiles], dt.bfloat16, kind="ExternalInput")
    iota_d = nc.dram_tensor("iota", [128, 128], dt.bfloat16, kind="ExternalInput")
    w1_d = nc.dram_tensor("w1", [256, 256], dt.bfloat16, kind="ExternalInput")
    w2_d = nc.dram_tensor("w2", [256, 256], dt.bfloat16, kind="ExternalInput")
    w3_d = nc.dram_tensor("w3", [256, 128], dt.bfloat16, kind="ExternalInput")
    w4_d = nc.dram_tensor("w4", [128, 2], dt.bfloat16, kind="ExternalInput")
    b1_d = nc.dram_tensor("b1", [1, 256], dt.bfloat16, kind="ExternalInput")
    b2_d = nc.dram_tensor("b2", [1, 256], dt.bfloat16, kind="ExternalInput")
    b3_d = nc.dram_tensor("b3", [1, 128], dt.bfloat16, kind="ExternalInput")
    b4_d = nc.dram_tensor("b4", [1, 2], dt.bfloat16, kind="ExternalInput")
    tok_d = nc.dram_tensor("tok", [128, 4], dt.float32, kind="ExternalInput")

    outp = nc.dram_tensor("outp", [2, NLOC], dt.float32, kind="ExternalOutput")
    tok_out = nc.dram_tensor("tok_out", [128, 4], dt.float32, kind="ExternalOutput")

    h1_tab = nc.dram_tensor("h1_tab", [NTAB, D], dt.bfloat16, addr_space="Shared")
    h2_tab = nc.dram_tensor("h2_tab", [NTAB, D], dt.bfloat16, addr_space="Shared")
    h3_tab = nc.dram_tensor("h3_tab", [NTAB, 128], dt.bfloat16, addr_space="Shared")
    sh1 = [nc.dram_tensor(f"sh1_{c}", [CHUNK, D], dt.bfloat16) for c in range(NCHUNK)]
    sh2 = [nc.dram_tensor(f"sh2_{c}", [CHUNK, D], dt.bfloat16) for c in range(NCHUNK)]
    sh3 = [nc.dram_tensor(f"sh3_{c}", [CHUNK, 128], dt.bfloat16) for c in range(NCHUNK)]

    RG = [list(range(N_CORES))]
    AF = mybir.ActivationFunctionType

    with tile.TileContext(nc) as tc:
        with tc.tile_pool(name="const", bufs=1) as cpool, \
             tc.tile_pool(name="msgp", bufs=2) as msgp, \
             tc.tile_pool(name="idxp", bufs=2) as idxp, \
             tc.tile_pool(name="indp", bufs=4) as indp, \
             tc.tile_pool(name="accp", bufs=8) as accp, \
             tc.tile_pool(name="work", bufs=4) as work, \
             tc.tile_pool(name="psA", bufs=4, space="PSUM") as psA, \
             tc.tile_pool(name="psB", bufs=2, space="PSUM") as psB, \
             tc.tile_pool(name="psO", bufs=2, space="PSUM") as psO:

            slot_all = cpool.tile([128, tot_tiles], dt.bfloat16)
            nc.sync.dma_start(out=slot_all[:], in_=slots_d[:])
            iota_t = cpool.tile([128, 128], dt.bfloat16)
            nc.sync.dma_start(out=iota_t[:], in_=iota_d[:])
            w1_t = cpool.tile([128, 2, 256], dt.bfloat16)
            nc.sync.dma_start(out=w1_t[:, 0, :], in_=w1_d[0:128, :])
            nc.sync.dma_start(out=w1_t[:, 1, :], in_=w1_d[128:256, :])
            w2_t = cpool.tile([128, 2, 256], dt.bfloat16)
            nc.sync.dma_start(out=w2_t[:, 0, :], in_=w2_d[0:128, :])
            nc.sync.dma_start(out=w2_t[:, 1, :], in_=w2_d[128:256, :])
            w3_t = cpool.tile([128, 2, 128], dt.bfloat16)
            nc.sync.dma_start(out=w3_t[:, 0, :], in_=w3_d[0:128, :])
            nc.sync.dma_start(out=w3_t[:, 1, :], in_=w3_d[128:256, :])
            w4_t = cpool.tile([128, 2], dt.bfloat16)
            nc.sync.dma_start(out=w4_t[:], in_=w4_d[:])
            b1_t = cpool.tile([1, 256], dt.bfloat16)
            nc.sync.dma_start(out=b1_t[:], in_=b1_d[:])
            b2_t = cpool.tile([1, 256], dt.bfloat16)
            nc.sync.dma_start(out=b2_t[:], in_=b2_d[:])
            b3_t = cpool.tile([1, 128], dt.bfloat16)
            nc.sync.dma_start(out=b3_t[:], in_=b3_d[:])
            b4_t = cpool.tile([1, 2], dt.bfloat16)
            nc.sync.dma_start(out=b4_t[:], in_=b4_d[:])
            ones_t = cpool.tile([1, 128], dt.bfloat16)
            nc.vector.memset(ones_t[:], 1.0)
            tok_t = cpool.tile([128, 4], dt.float32)
            nc.sync.dma_start(out=tok_t[:], in_=tok_d[:])

            def gather_group(g, tab_ap, Dl):
                """Gather all edges of group g: msgs [128, g_tiles, Dl]."""
                gt0 = int(g_t0[g])
                gtiles = int(g_len[g]) // 128
                cols = int(g_len[g]) // 16
                c0 = int(g_off[g]) // 16
                idxg = idxp.tile([128, cols], dt.int16, tag="idxg")
                nc.sync.dma_start(out=idxg[:], in_=idxs_d[:, c0:c0 + cols])
                msgs = msgp.tile([128, gtiles, Dl], dt.bfloat16, tag="msgs")
                for qq in range(4):
                    ni = int(gq_len[g, qq])
                    if ni == 0:
                        continue
                    o16 = int(gq_off[g, qq] - g_off[g]) // 16
                    to = int(gq_t0[g, qq]) - gt0
                    ntq = ni // 128
                    nc.gpsimd.dma_gather(
                        msgs[:, to:to + ntq, :],
                        tab_ap[qq * QROWS:(qq + 1) * QROWS, :],
                        idxg[:, o16:o16 + ni // 16],
                        ni, ni, Dl, single_packet=False)
                return msgs, gt0

            def block_pairs(b, gt0):
                """(group-tile index, block-tile index) pairs for block b."""
                pairs = []
                bo = 0
                for qq in range(4):
                    ntq = int(nt_bq[b, qq])
                    gt = int(tile_of[b, qq]) - gt0
                    for k in range(ntq):
                        pairs.append((gt + k, bo + k))
                    bo += ntq
                return pairs

            def block_ind(b):
                """One-hot indicators for block b's tiles."""
                ntb = int(nt_blk[b])
                ind = indp.tile([128, ntb, 128], dt.bfloat16, tag="ind")
                bo = 0
                for qq in range(4):
                    ntq = int(nt_bq[b, qq])
                    if ntq == 0:
                        continue
                    t0 = int(tile_of[b, qq])
                    nc.vector.tensor_tensor(
                        out=ind[:, bo:bo + ntq, :],
                        in0=slot_all[:, t0:t0 + ntq][:, :, None]
                            .to_broadcast([128, ntq, 128]),
                        in1=iota_t[:][:, None, :].to_broadcast([128, ntq, 128]),
                        op=mybir.AluOpType.is_equal)
                    bo += ntq
                return ind

            def agg_blockT(b, msgs, gt0, nhalf):
                """accT[f_half, k, d] += msgs^T @ ind, one chain per half."""
                ind = block_ind(b)
                pairs = block_pairs(b, gt0)
                acc = psA.tile([128, nhalf, 128], dt.float32, space="PSUM",
                               tag="acc")
                for k in range(nhalf):
                    for i, (mt, it) in enumerate(pairs):
                        nc.tensor.matmul(
                            out=acc[:, k, :],
                            lhsT=msgs[:, mt, k * 128:(k + 1) * 128],
                            rhs=ind[:, it, :],
                            start=(i == 0), stop=(i == len(pairs) - 1))
                return acc

            def tail_linear(b, acc, nhalf, w_t, b_t, nout, relu, h_shards,
                            tab):
                """h[d, nout] = (relu?)(sum_k accT_k^T @ W_k + b); store."""
                a_sb = accp.tile([128, nhalf, 128], dt.bfloat16, tag="a_sb")
                for k in range(nhalf):
                    nc.vector.tensor_copy(out=a_sb[:, k, :], in_=acc[:, k, :])
                hps = psB.tile([128, nout], dt.float32, space="PSUM", tag="h")
                for k in range(nhalf):
                    nc.tensor.matmul(out=hps[:], lhsT=a_sb[:, k, :],
                                     rhs=w_t[:, k, :], start=(k == 0),
                                     stop=False)
                nc.tensor.matmul(out=hps[:], lhsT=ones_t[:], rhs=b_t[:],
                                 start=False, stop=True)
                h_blk = work.tile([128, nout], dt.bfloat16, tag="hblk")
                if relu:
                    nc.scalar.activation(out=h_blk[:], in_=hps[:], func=AF.Relu)
                else:
                    nc.vector.tensor_copy(out=h_blk[:], in_=hps[:])
                r0 = (b % BPC) * BLK
                nc.sync.dma_start(out=h_shards[b // BPC][r0:r0 + BLK, :],
                                  in_=h_blk[:])
                if collectives and b % BPC == BPC - 1:
                    c = b // BPC
                    nc.gpsimd.collective_compute(
                        "AllGather", mybir.AluOpType.bypass, replica_groups=RG,
                        ins=[h_shards[c][:]],
                        outs=[tab[c * N_CORES * CHUNK:(c + 1) * N_CORES * CHUNK, :]])

            def tail_out(b, acc):
                """out rows = W4^T @ accT + b4; store [2, 128] columns."""
                a_sb = accp.tile([128, 1, 128], dt.bfloat16, tag="a_sb")
                nc.vector.tensor_copy(out=a_sb[:, 0, :], in_=acc[:, 0, :])
                ops = psO.tile([2, 128], dt.float32, space="PSUM", tag="o")
                nc.tensor.matmul(out=ops[:], lhsT=w4_t[:], rhs=a_sb[:, 0, :],
                                 start=True, stop=False)
                nc.tensor.matmul(out=ops[:], lhsT=b4_t[:], rhs=ones_t[:],
                                 start=False, stop=True)
                ob = work.tile([2, 128], dt.float32, tag="ob")
                nc.vector.tensor_copy(out=ob[:], in_=ops[:])
                nc.sync.dma_start(out=outp[:, b * BLK:(b + 1) * BLK], in_=ob[:])

            def stream_group(g, Dl):
                """L1: bulk-load the host-pregathered edge stream slice."""
                gt0 = int(g_t0[g])
                gtiles = int(g_len[g]) // 128
                msgs = msgp.tile([128, gtiles, Dl], dt.bfloat16, tag="msgs")
                nc.sync.dma_start(out=msgs[:],
                                  in_=x_edges_d[:, gt0:gt0 + gtiles, :])
                return msgs, gt0

            def run_layer(tab_in, Dl, tail_fn, stream=False):
                """Software-pipelined: aggs of group g, then tails of g-1."""
                nhalf = Dl // 128
                pending = []
                for g in range(NGRP):
                    if stream:
                        msgs, gt0 = stream_group(g, Dl)
                    else:
                        msgs, gt0 = gather_group(g, tab_in, Dl)
                    new_pending = []
                    for b in grp_blocks[g]:
                        acc = agg_blockT(b, msgs, gt0, nhalf)
                        new_pending.append((b, acc))
                    for b, acc in pending:
                        tail_fn(b, acc)
                    pending = new_pending
                for b, acc in pending:
                    tail_fn(b, acc)

            for rep in range(repeat):
                run_layer(None, D, lambda b, acc: tail_linear(
                    b, acc, 2, w1_t, b1_t, 256, True, sh1, h1_tab), stream=True)
                if layers >= 2:
                    run_layer(h1_tab, D, lambda b, acc: tail_linear(
                        b, acc, 2, w2_t, b2_t, 256, True, sh2, h2_tab))
                if layers >= 3:
                    run_layer(h2_tab, D, lambda b, acc: tail_linear(
                        b, acc, 2, w3_t, b3_t, 128, True, sh3, h3_tab))
                if layers >= 4:
                    run_layer(h3_tab, 128, tail_out)

            # token passthrough (anti-CSE for timing harness)
            nc.scalar.mul(tok_t[:], tok_t[:], 2.0)
            nc.sync.dma_start(out=tok_out[:], in_=tok_t[:])

    nc.compile()
    return nc


def make_in_maps(x, src, dst, W1, b1, W2, b2, W3, b3, W4, b4,
                 meta, idx_streams, slot_streams, tid_streams):
    v = np.arange(N)
    tid = _table_id(v)
    x_tab = np.zeros((NTAB, D), bf16)
    x_tab[tid] = np.asarray(x).astype(bf16)
    tot_tiles = int(meta['tot_tiles'])
    iota_np = np.tile(np.arange(128, dtype=np.float32)[None, :], (128, 1)).astype(bf16)
    common = {
        "iota": iota_np,
        "w1": np.asarray(W1).astype(bf16), "w2": np.asarray(W2).astype(bf16),
        "w3": np.asarray(W3).astype(bf16), "w4": np.asarray(W4).astype(bf16),
        "b1": np.asarray(b1).reshape(1, -1).astype(bf16),
        "b2": np.asarray(b2).reshape(1, -1).astype(bf16),
        "b3": np.asarray(b3).reshape(1, -1).astype(bf16),
        "b4": np.asarray(b4).reshape(1, -1).astype(bf16),
        "tok": np.zeros((128, 4), np.float32),
    }
    in_maps = []
    for c in range(N_CORES):
        m = dict(common)
        m["idxs"] = idx_streams[c]
        m["slots"] = slot_streams[c]
        # host-pregathered layer-1 edge stream: position i -> [i%128, i//128]
        m["x_edges"] = x_tab[tid_streams[c].reshape(tot_tiles, 128).T]
        in_maps.append(m)
    return in_maps


def assemble_output(results):
    out = np.zeros((N, OUTD), np.float32)
    for c in range(N_CORES):
        o = results[c]["outp"]            # [2, NLOC]
        out[c * RPC:(c + 1) * RPC, :] = o.T[:RPC, :]
    return out


_CACHE = {}
LAST = {}


def kernel(x, src, dst, W1, b1, W2, b2, W3, b3, W4, b4):
    src = np.asarray(src)
    dst = np.asarray(dst)
    key = (src.tobytes(), dst.tobytes())
    kh = hash(key)
    if kh in _CACHE:
        meta, idx_streams, slot_streams, tid_streams, nc = _CACHE[kh]
    else:
        meta, idx_streams, slot_streams, tid_streams = preprocess(src, dst)
        nc = build_program(meta)
        _CACHE[kh] = (meta, idx_streams, slot_streams, tid_streams, nc)
    in_maps = make_in_maps(x, src, dst, W1, b1, W2, b2, W3, b3, W4, b4,
                           meta, idx_streams, slot_streams, tid_streams)
    LAST.update(nc=nc, in_maps=in_maps, meta=meta)
    res = bass_utils.run_bass_kernel_spmd(nc, in_maps, core_ids=list(range(N_CORES)))
    return assemble_output(res.results)
